# revision 10
# baseline (speedup 1.0000x reference)
"""Bass/Trainium2 kernel for nn_Attention_75007308857927 (v3).

Reference computation (B=4, S=2048, D=1024):
    Q = X @ Wq.T ; K = X @ Wk.T ; V = X @ Wv.T         (per batch)
    Qn, Kn = row-normalized Q, K
    scores = (Qn @ Kn.T) * m      m[i,j] = 1 if (j > i) or masks[j]==0 else 0
    out = scores @ V

Sharding: 2 cores per batch; each core owns 1024 reordered keys (padded
asc ++ valid asc, host-side) and 1024 queries (OWN_TILES interleave).
KT and V' are pair-AllGathered; scores/out computed per own queries with
anti-causal block skipping (bounds derived from the actual masks,
SPMD-uniform).

v3 changes over v2 (243us -> target ~195us):
  * DMA issue rebalanced around two measured facts: a dma_start costs
    ~0.6-0.8us of sequencer time, and an engine issuing a large DMA
    queue is credit-blocked until most of it drains. The scalar (ACT)
    engine now carries NO early DMAs so the projection squares start on
    time; gpsimd carries xk/wvt/xq + all cc stores + doorbells; sync
    carries wkt/xk-hi/wqt/masks + gather loads.
  * k/q sumsq: squares on ACT, et-chunk accumulation on DVE (sequential
    bf16 adds), so the PE does only 2 ones-matmuls + 8 transposes per
    tensor (was 16 ones-matmuls each, and they stalled the in-order PE
    stream behind a credit-blocked scalar engine).
  * masks shipped as a single fp8 (0/1-exact) tile, one DMA.
  * output stored bf16 (host converts to f32); halves the tail store.

bf16 matmul operands, f32 PSUM accumulation.
"""

import numpy as np
import ml_dtypes

B, S, D = 4, 2048, 1024
HALF = S // 2          # queries / keys per core
N_CORES = 8
P = 128
DC = D // P            # 8 contraction chunks over d
ET = D // P            # 8 e-tiles
JT = S // P            # 16 global key tiles
JTH = HALF // P        # 8 own key tiles

BF16 = ml_dtypes.bfloat16
FP8 = ml_dtypes.float8_e4m3

# query-tile assignment: balanced anti-causal load, sorted ascending
OWN_TILES = [
    [0, 2, 4, 6, 9, 11, 13, 15],   # h = 0
    [1, 3, 5, 7, 8, 10, 12, 14],   # h = 1
]

_CACHE = {}


def _emit(ctx, tc, cfg, xq, xk, wkt, wvt, wqt, maskt, out, cc):
    from concourse import mybir

    JA, TCOPY, GSTART = cfg
    FSTART = (min(GSTART[0:4]), min(GSTART[4:8]))
    f_tiles = [list(range(JA)) + list(range(FSTART[s], JT)) for s in range(2)]
    st_pos = [{t: i for i, t in enumerate(f_tiles[s])} for s in range(2)]
    NT = [len(f_tiles[s]) for s in range(2)]
    mrow = {}
    r = 0
    for s in range(2):
        for t in f_tiles[s]:
            if t >= TCOPY:
                mrow[(s, t)] = r
                r += 1
    NM = r

    nc = tc.nc
    dtb = mybir.dt.bfloat16
    dtf = mybir.dt.float32
    dt8 = mybir.dt.float8e4

    # ---- SBUF pools -------------------------------------------------------
    big_p = ctx.enter_context(tc.tile_pool(name="big", bufs=4))
    xq_p = ctx.enter_context(tc.tile_pool(name="xq", bufs=1))
    qt_p = ctx.enter_context(tc.tile_pool(name="qt", bufs=1))
    kt_p = ctx.enter_context(tc.tile_pool(name="kt", bufs=1))
    vp_p = ctx.enter_context(tc.tile_pool(name="vp", bufs=1))
    row_p = ctx.enter_context(tc.tile_pool(name="rows", bufs=1))
    sq_p = ctx.enter_context(tc.tile_pool(name="sq", bufs=9))
    red_p = ctx.enter_context(tc.tile_pool(name="red", bufs=2))
    stg_p = ctx.enter_context(tc.tile_pool(name="stg", bufs=6))
    mk_p = ctx.enter_context(tc.tile_pool(name="mk", bufs=1))
    ev_p = ctx.enter_context(tc.tile_pool(name="ev", bufs=4))
    # 7 main matmul banks + 1 bank for the row/col reductions
    ps_p = ctx.enter_context(tc.tile_pool(name="psmm", bufs=7, space="PSUM"))
    psr_p = ctx.enter_context(tc.tile_pool(name="psrow", bufs=1, space="PSUM"))

    wkt_s = big_p.tile([P, DC * D], dtb, tag="big")    # [d%128, dc*1024+e]
    wvt_s = big_p.tile([P, DC * D], dtb, tag="big")
    wqt_s = big_p.tile([P, DC * D], dtb, tag="big")
    xk_s = big_p.tile([P, DC * HALF], dtb, tag="big")  # [d%128, dc*1024+p']
    xq_s = xq_p.tile([P, DC * HALF], dtb, tag="xq")    # [d%128, dc*1024+i]
    qt_s = qt_p.tile([P, ET * HALF], dtb, tag="qt")    # [e%128, et*1024+i]
    kt_s = kt_p.tile([P, ET * S], dtb, tag="kt")       # [e%128, et*2048+p]
    vp_s = vp_p.tile([P, JT * D], dtb, tag="vp")       # [p%128, pt*1024+d]
    mk_s = mk_p.tile([P, max(NM, 1) * 512], dt8, tag="mk")

    ones_b = row_p.tile([P, 1], dtb, tag="ones_b")
    ones_f = row_p.tile([1, 1], dtb, tag="ones_f")
    ksq_row = row_p.tile([1, HALF], dtb, tag="sqrow")
    qsq_row = row_p.tile([1, HALF], dtb, tag="sqrow2")
    ksq_col = row_p.tile([P, JTH], dtf, tag="ksqc")
    krec_col = row_p.tile([P, JTH], dtf, tag="krecc")
    kinv_col = row_p.tile([P, JTH], dtf, tag="kinvc")
    qsq_col = row_p.tile([P, ET], dtf, tag="qsqc")
    qrec_col = row_p.tile([P, ET], dtf, tag="qrecc")
    qinv_col = row_p.tile([P, ET], dtf, tag="qinvc")

    warm = row_p.tile([P, 512], dtb, tag="warm")
    nc.vector.memset(ones_b[:], 1.0)
    nc.vector.memset(ones_f[:], 1.0)
    nc.vector.memset(warm[:], 0.5)

    groups = [[0, 1], [2, 3], [4, 5], [6, 7]]

    # PE warm-up: ramp out of the low p-state while input DMAs land
    ps_warm = psr_p.tile([1, 512], mybir.dt.float32, tag="psrow", name="warm")
    for _ in range(10):
        nc.tensor.matmul(ps_warm[:], lhsT=ones_b[:], rhs=warm[:],
                         start=True, stop=True)

    # ---- input DMAs -------------------------------------------------------
    # sync: wkt and xk j5=0 interleaved in wave-1 consumption order (ring
    #       FIFO = issue order), then xk j5=1; later kt-gather loads + even
    #       out stores. gpsimd/scalar issue nothing before ~25us so these
    #       descriptors are never queued behind lower-priority bytes.
    wkt_s3 = wkt_s[:].rearrange("p (dc e) -> p dc e", dc=DC)
    wkt3d = wkt.rearrange("(dc p) e -> p dc e", p=P)
    xk_s4 = xk_s[:].rearrange("p (dc h j) -> p dc h j", dc=DC, h=2)
    xk4d = xk.rearrange("(dc p) (h j) -> p dc h j", p=P, h=2)
    nc.sync.dma_start(wkt_s3[:, 0, 0:256], wkt3d[:, 0, 0:256])
    nc.sync.dma_start(xk_s4[:, 0:4, 0, :], xk4d[:, 0:4, 0, :])
    nc.sync.dma_start(wkt_s3[:, 0, 256:1024], wkt3d[:, 0, 256:1024])
    nc.sync.dma_start(wkt_s3[:, 1, :], wkt3d[:, 1, :])
    nc.sync.dma_start(xk_s4[:, 4:8, 0, :], xk4d[:, 4:8, 0, :])
    nc.sync.dma_start(wkt_s3[:, 2:4, :], wkt3d[:, 2:4, :])
    nc.sync.dma_start(wkt_s3[:, 4:6, :], wkt3d[:, 4:6, :])
    nc.sync.dma_start(wkt_s3[:, 6:8, :], wkt3d[:, 6:8, :])
    nc.sync.dma_start(xk_s4[:, 0:4, 1, :], xk4d[:, 0:4, 1, :])
    nc.sync.dma_start(xk_s4[:, 4:8, 1, :], xk4d[:, 4:8, 1, :])

    wvt3d = wvt.rearrange("(dc p) e -> p dc e", p=P)
    wvt_s3 = wvt_s[:].rearrange("p (dc e) -> p dc e", dc=DC)
    wqt3d = wqt.rearrange("(dc p) e -> p dc e", p=P)
    wqt_s3 = wqt_s[:].rearrange("p (dc e) -> p dc e", dc=DC)
    mk3 = mk_s[:].rearrange("p (r i) -> p r i", r=max(NM, 1))
    m3 = maskt.rearrange("(r p) i -> p r i", p=P)

    # ---- phase B: KT = Wk^T X for own keys + k sumsq ----------------------
    sq_k = []

    def b_evict(j5, et, ps):
        stg = stg_p.tile([P, 512], dtb, tag="stg", name=f"stgb{j5}_{et}")
        nc.vector.tensor_copy(stg[:], ps[:])
        nc.gpsimd.dma_start(cc[f"kt_own{j5}"][et * P:(et + 1) * P, :], stg[:])
        sq = sq_p.tile([P, 512], dtb, tag="sq", name=f"sqb{j5}_{et}")
        nc.scalar.square(sq[:], stg[:])
        sq_k.append(sq)

    acc_k = [red_p.tile([P, 512], dtb, tag="red", name=f"acck{j5}")
             for j5 in range(2)]

    def b_wave(j5):
        # et0-6 dc-outer across 7 banks, then et7 dc-inner
        ps_wave = [ps_p.tile([P, 512], dtf, tag="psmm", name=f"bw{j5}_{et}")
                   for et in range(7)]
        for dc in range(DC):
            for et in range(7):
                nc.tensor.matmul(
                    ps_wave[et][:],
                    lhsT=wkt_s[:, dc * D + et * P: dc * D + (et + 1) * P],
                    rhs=xk_s[:, dc * HALF + j5 * 512: dc * HALF + j5 * 512 + 512],
                    start=(dc == 0), stop=(dc == DC - 1),
                )
        for et in range(7):
            b_evict(j5, et, ps_wave[et])
        ps7 = ps_p.tile([P, 512], dtf, tag="psmm", name=f"bx{j5}")
        for dc in range(DC):
            nc.tensor.matmul(
                ps7[:],
                lhsT=wkt_s[:, dc * D + 7 * P: dc * D + 8 * P],
                rhs=xk_s[:, dc * HALF + j5 * 512: dc * HALF + j5 * 512 + 512],
                start=(dc == 0), stop=(dc == DC - 1),
            )
        b_evict(j5, 7, ps7)
        # DVE sumsq accumulation over the 8 et chunks
        base = 8 * j5
        nc.vector.tensor_add(acc_k[j5][:], sq_k[base][:], sq_k[base + 1][:])
        for et in range(2, 8):
            nc.vector.tensor_add(acc_k[j5][:], acc_k[j5][:], sq_k[base + et][:])

    b_wave(0)
    nc.gpsimd.collective_compute(
        "AllGather", mybir.AluOpType.bypass, replica_groups=groups,
        ins=[cc["kt_own0"][:]], outs=[cc["kt_gath0"][:]])
    # wvt now: gpsimd's stream reaches this ~25us in (after the j5=0 kt
    # stores), so wvt's 2MB never contends with the wave-1/2 inputs
    nc.gpsimd.dma_start(wvt_s3[:, 0:4, :], wvt3d[:, 0:4, :])
    nc.gpsimd.dma_start(wvt_s3[:, 4:8, :], wvt3d[:, 4:8, :])
    b_wave(1)
    nc.gpsimd.collective_compute(
        "AllGather", mybir.AluOpType.bypass, replica_groups=groups,
        ins=[cc["kt_own1"][:]], outs=[cc["kt_gath1"][:]])

    # xq on gpsimd (needed only by phase E); wqt + masks on scalar, whose
    # stream reaches this point only after the wave-1 squares (~35us)
    xq3d = xq.rearrange("(dc p) e -> p dc e", p=P)
    xq_s3 = xq_s[:].rearrange("p (dc e) -> p dc e", dc=DC)
    nc.gpsimd.dma_start(xq_s3[:, 0:4, :], xq3d[:, 0:4, :])
    nc.gpsimd.dma_start(xq_s3[:, 4:8, :], xq3d[:, 4:8, :])
    nc.scalar.dma_start(wqt_s3[:, 0:4, :], wqt3d[:, 0:4, :])
    nc.scalar.dma_start(wqt_s3[:, 4:8, :], wqt3d[:, 4:8, :])
    nc.scalar.dma_start(mk3[:, :, :], m3[:, :, :])

    # k sumsq -> kinv: 2 ones-matmuls + 8 transposes on PE, recip on DVE,
    # sqrt on ACT
    for j5 in range(2):
        pr = psr_p.tile([1, 512], dtf, tag="psrow", name=f"ksqps{j5}")
        nc.tensor.matmul(pr[:], lhsT=ones_b[:], rhs=acc_k[j5][:],
                         start=True, stop=True)
        nc.vector.tensor_copy(ksq_row[0:1, j5 * 512:(j5 + 1) * 512], pr[:])
    for c in range(JTH):
        pc = psr_p.tile([P, 1], dtf, tag="psrow", name=f"kpc{c}")
        nc.tensor.matmul(pc[:], lhsT=ksq_row[0:1, c * P:(c + 1) * P],
                         rhs=ones_f[:], start=True, stop=True)
        nc.vector.tensor_copy(ksq_col[:, c:c + 1], pc[:])
    nc.vector.reciprocal(krec_col[:], ksq_col[:])
    nc.scalar.sqrt(kinv_col[:], krec_col[:])

    # ---- phase D: V' = V * kinv for own keys ------------------------------
    for jt in range(JTH):
        ps_a = ps_p.tile([P, 512], dtf, tag="psmm")
        ps_b = ps_p.tile([P, 512], dtf, tag="psmm")
        pspair = [ps_a, ps_b]
        for dc in range(DC):
            for e5 in range(2):
                nc.tensor.matmul(
                    pspair[e5][:],
                    lhsT=xk_s[:, dc * HALF + jt * P: dc * HALF + (jt + 1) * P],
                    rhs=wvt_s[:, dc * D + e5 * 512: dc * D + e5 * 512 + 512],
                    start=(dc == 0), stop=(dc == DC - 1),
                )
        half = jt // 4
        for e5 in range(2):
            stg = stg_p.tile([P, 512], dtb, tag="stg", name=f"stgv{jt}_{e5}")
            nc.vector.tensor_scalar_mul(stg[:], pspair[e5][:],
                                        kinv_col[:, jt:jt + 1])
            nc.gpsimd.dma_start(
                cc[f"v_own{half}"][(jt % 4) * P:(jt % 4 + 1) * P,
                                   e5 * 512: e5 * 512 + 512], stg[:])
        if jt == 3:
            nc.gpsimd.collective_compute(
                "AllGather", mybir.AluOpType.bypass, replica_groups=groups,
                ins=[cc["v_own0"][:]], outs=[cc["v_gath0"][:]])
    nc.gpsimd.collective_compute(
        "AllGather", mybir.AluOpType.bypass, replica_groups=groups,
        ins=[cc["v_own1"][:]], outs=[cc["v_gath1"][:]])

    # kt SBUF loads (1MB per (half, rank)) on sync
    kt3 = kt_s[:].rearrange("p (et j) -> p et j", et=ET, j=S)
    for h5 in range(2):
        gath = cc[f"kt_gath{h5}"]
        for r in range(2):
            src3 = gath[r].rearrange("(et p) j -> p et j", p=P)
            dst = kt3[:, :, r * HALF + h5 * 512: r * HALF + h5 * 512 + 512]
            nc.sync.dma_start(dst, src3)

    # vp SBUF loads on gpsimd
    vp3 = vp_s[:].rearrange("p (jtl e) -> p jtl e", jtl=JT, e=D)
    for h5 in range(2):
        gath = cc[f"v_gath{h5}"]
        for r in range(2):
            src_ap = gath[r].rearrange("(jtl p) e -> p jtl e", p=P)
            dst = vp3[:, r * JTH + h5 * 4: r * JTH + h5 * 4 + 4, :]
            nc.gpsimd.dma_start(dst, src_ap)

    # ---- phase E: QT = Wq^T X for own queries + q sumsq -------------------
    sq_q = []

    def e_evict(i5, et, ps):
        qtsl = qt_s[:, et * HALF + i5 * 512: et * HALF + i5 * 512 + 512]
        nc.vector.tensor_copy(qtsl, ps[:])
        sq = sq_p.tile([P, 512], dtb, tag="sq", name=f"sqe{i5}_{et}")
        nc.scalar.square(sq[:], qtsl)
        sq_q.append(sq)

    acc_q = [red_p.tile([P, 512], dtb, tag="red", name=f"accq{i5}")
             for i5 in range(2)]

    def e_wave(i5):
        ps_wave = [ps_p.tile([P, 512], dtf, tag="psmm", name=f"ew{i5}_{et}")
                   for et in range(7)]
        for dc in range(DC):
            for et in range(7):
                nc.tensor.matmul(
                    ps_wave[et][:],
                    lhsT=wqt_s[:, dc * D + et * P: dc * D + (et + 1) * P],
                    rhs=xq_s[:, dc * HALF + i5 * 512: dc * HALF + i5 * 512 + 512],
                    start=(dc == 0), stop=(dc == DC - 1),
                )
        for et in range(7):
            e_evict(i5, et, ps_wave[et])
        ps7 = ps_p.tile([P, 512], dtf, tag="psmm", name=f"ex{i5}")
        for dc in range(DC):
            nc.tensor.matmul(
                ps7[:],
                lhsT=wqt_s[:, dc * D + 7 * P: dc * D + 8 * P],
                rhs=xq_s[:, dc * HALF + i5 * 512: dc * HALF + i5 * 512 + 512],
                start=(dc == 0), stop=(dc == DC - 1),
            )
        e_evict(i5, 7, ps7)
        base = 8 * i5
        nc.vector.tensor_add(acc_q[i5][:], sq_q[base][:], sq_q[base + 1][:])
        for et in range(2, 8):
            nc.vector.tensor_add(acc_q[i5][:], acc_q[i5][:], sq_q[base + et][:])

    e_wave(0)
    e_wave(1)

    # q sumsq -> qinv (qinv only needed by phase G evictions)
    for i5 in range(2):
        pr = psr_p.tile([1, 512], dtf, tag="psrow", name=f"qsqps{i5}")
        nc.tensor.matmul(pr[:], lhsT=ones_b[:], rhs=acc_q[i5][:],
                         start=True, stop=True)
        nc.vector.tensor_copy(qsq_row[0:1, i5 * 512:(i5 + 1) * 512], pr[:])
    for c in range(ET):
        pc = psr_p.tile([P, 1], dtf, tag="psrow", name=f"qpc{c}")
        nc.tensor.matmul(pc[:], lhsT=qsq_row[0:1, c * P:(c + 1) * P],
                         rhs=ones_f[:], start=True, stop=True)
        nc.vector.tensor_copy(qsq_col[:, c:c + 1], pc[:])
    nc.vector.reciprocal(qrec_col[:], qsq_col[:])
    nc.scalar.sqrt(qinv_col[:], qrec_col[:])

    # ---- phase F: score blocks (dense padded part + anti-causal part) -----
    st_blks = [big_p.tile([P, NT[s] * 512], dtb, tag="big", name=f"st_blk{s}")
               for s in range(2)]
    # consume the gather-0 half (local tiles 0-3, 8-11) first: its SBUF
    # loads land ~20us before gather-1's
    f_order = [sorted(f_tiles[s], key=lambda t: ((t // 4) % 2, t))
               for s in range(2)]
    for s in range(2):
        st_blk = st_blks[s]
        for t in f_order[s]:
            pos = st_pos[s][t]
            ps = ps_p.tile([P, 512], dtf, tag="psmm")
            for et in range(ET):
                nc.tensor.matmul(
                    ps[:],
                    lhsT=kt_s[:, et * S + t * P: et * S + (t + 1) * P],
                    rhs=qt_s[:, et * HALF + s * 512: et * HALF + s * 512 + 512],
                    start=(et == 0), stop=(et == ET - 1),
                )
            dst = st_blk[:, pos * 512:(pos + 1) * 512]
            if t < TCOPY:
                nc.vector.tensor_copy(dst, ps[:])
            else:
                rr = mrow[(s, t)]
                nc.vector.tensor_mul(dst, ps[:], mk3[:, rr, :])

    # ---- phase G: out = (ST^T @ V') * qinv, stored bf16 -------------------
    for s in range(2):
        st_blk = st_blks[s]
        for c in range(4):
            k = 4 * s + c
            g_tiles = list(range(JA)) + list(range(GSTART[k], JT))
            # accumulate v_gath0's tiles first so the chain only stalls on
            # the late v_gath1 loads near its end
            g_tiles.sort(key=lambda t: ((t // 4) % 2, t))
            ev = ev_p.tile([P, D], dtb, tag="ev", name=f"ev{k}")
            for d5 in range(2):
                ps = ps_p.tile([P, 512], dtf, tag="psmm")
                for n, t in enumerate(g_tiles):
                    pos = st_pos[s][t]
                    nc.tensor.matmul(
                        ps[:],
                        lhsT=st_blk[:, pos * 512 + c * P: pos * 512 + (c + 1) * P],
                        rhs=vp_s[:, t * D + d5 * 512: t * D + d5 * 512 + 512],
                        start=(n == 0), stop=(n == len(g_tiles) - 1),
                    )
                nc.vector.tensor_scalar_mul(ev[:, d5 * 512:d5 * 512 + 512],
                                            ps[:], qinv_col[:, k:k + 1])
            eng = nc.sync if k % 2 == 0 else nc.scalar
            eng.dma_start(out[k * P:(k + 1) * P, :], ev[:])


def _build(cfg):
    if cfg in _CACHE:
        return _CACHE[cfg]
    import concourse.tile as tile
    from concourse import bacc, mybir

    JA, TCOPY, GSTART = cfg
    FSTART = (min(GSTART[0:4]), min(GSTART[4:8]))
    f_tiles = [list(range(JA)) + list(range(FSTART[s], JT)) for s in range(2)]
    NM = sum(sum(1 for t in f_tiles[s] if t >= TCOPY) for s in range(2))

    dtb = mybir.dt.bfloat16
    dt8 = mybir.dt.float8e4
    nc = bacc.Bacc("TRN2", target_bir_lowering=False, debug=False,
                   enable_asserts=True, num_devices=N_CORES)
    xq = nc.dram_tensor("xq", [D, HALF], dtb, kind="ExternalInput").ap()
    xk = nc.dram_tensor("xk", [D, HALF], dtb, kind="ExternalInput").ap()
    wkt = nc.dram_tensor("wkt", [D, D], dtb, kind="ExternalInput").ap()
    wvt = nc.dram_tensor("wvt", [D, D], dtb, kind="ExternalInput").ap()
    wqt = nc.dram_tensor("wqt", [D, D], dtb, kind="ExternalInput").ap()
    maskt = nc.dram_tensor("maskt", [max(NM, 1) * P, 512], dt8,
                           kind="ExternalInput").ap()
    out = nc.dram_tensor("out", [HALF, D], dtb, kind="ExternalOutput").ap()
    cc = {}
    for h5 in range(2):
        cc[f"kt_own{h5}"] = nc.dram_tensor(f"kt_own{h5}", [D, 512], dtb).ap()
        cc[f"kt_gath{h5}"] = nc.dram_tensor(f"kt_gath{h5}", [2, D, 512], dtb).ap()
        cc[f"v_own{h5}"] = nc.dram_tensor(f"v_own{h5}", [512, D], dtb).ap()
        cc[f"v_gath{h5}"] = nc.dram_tensor(f"v_gath{h5}", [2, 512, D], dtb).ap()

    from contextlib import ExitStack
    with tile.TileContext(nc) as tc:
        with ExitStack() as ctx:
            _emit(ctx, tc, cfg, xq, xk, wkt, wvt, wqt, maskt, out, cc)
    nc.compile()
    _CACHE[cfg] = nc
    return nc


def plan(masks):
    """Derive key reorder + skip bounds from the masks (SPMD-uniform)."""
    masks = np.asarray(masks)
    packed, nAs = [], []
    for b in range(B):
        iA = np.flatnonzero(masks[b] == 0)   # padded: always visible
        iV = np.flatnonzero(masks[b] != 0)   # valid: visible iff j > i
        packed.append(np.concatenate([iA, iV]).astype(np.int64))
        nAs.append(len(iA))
    JA = max(max(-(-n // P) for n in nAs), 1)
    TCOPY = min(n // P for n in nAs)
    GSTART = []
    for k in range(8):
        st = JT
        for b in range(B):
            pk, nA = packed[b], nAs[b]
            for h in range(2):
                g = OWN_TILES[h][k]
                t = JA
                while t < JT:
                    p = np.arange(t * P, (t + 1) * P)
                    mv = np.where(p >= nA, pk[p], -1).max()
                    if mv > g * P:
                        break
                    t += 1
                st = min(st, t)
        GSTART.append(st)
    # slots ascend in tile index -> bounds must be non-increasing suffixes
    for k in range(6, -1, -1):
        GSTART[k] = min(GSTART[k], GSTART[k + 1])
    return (JA, TCOPY, tuple(GSTART)), packed, nAs


def make_in_maps(X, masks, Wq, Wk, Wv):
    """Host-side key reorder + layout: one input map per core."""
    cfg, packed, nAs = plan(masks)
    JA, TCOPY, GSTART = cfg
    FSTART = (min(GSTART[0:4]), min(GSTART[4:8]))
    f_tiles = [list(range(JA)) + list(range(FSTART[s], JT)) for s in range(2)]
    NM = sum(sum(1 for t in f_tiles[s] if t >= TCOPY) for s in range(2))
    in_maps = []
    wkt_h = np.ascontiguousarray(Wk.T).astype(BF16)
    wvt_h = np.ascontiguousarray(Wv.T).astype(BF16)
    wqt_h = np.ascontiguousarray(Wq.T).astype(BF16)
    for c in range(N_CORES):
        b, h = c // 2, c % 2
        XT = X[b].T.astype(BF16)                                # [D, S]
        pk, nA = packed[b], nAs[b]
        own_keys = pk[h * HALF:(h + 1) * HALF]
        tiles = OWN_TILES[h]
        qrows = np.concatenate([np.arange(g * P, (g + 1) * P) for g in tiles])
        # mask blocks, [NM*128, 512]: rows = global reordered key pos,
        # cols = own queries in slot order
        mt = np.zeros((max(NM, 1) * P, 4 * P), FP8)
        cols = [np.concatenate([np.arange(tiles[4 * s + cc] * P,
                                          (tiles[4 * s + cc] + 1) * P)
                                for cc in range(4)]) for s in range(2)]
        r = 0
        for s in range(2):
            for t in f_tiles[s]:
                if t < TCOPY:
                    continue
                p = t * P + np.arange(P)
                is_pad = p < nA
                vis = is_pad[:, None] | (pk[p][:, None] > cols[s][None, :])
                mt[r * P:(r + 1) * P, :] = vis
                r += 1
        in_maps.append({
            "xq": np.ascontiguousarray(XT[:, qrows]),
            "xk": np.ascontiguousarray(XT[:, own_keys]),
            "wkt": wkt_h,
            "wvt": wvt_h,
            "wqt": wqt_h,
            "maskt": mt,
        })
    return in_maps, cfg


def run(in_maps, cfg, **kw):
    from concourse.bass_utils import run_bass_kernel_spmd
    nc = _build(cfg)
    return run_bass_kernel_spmd(nc, in_maps, list(range(N_CORES)), **kw)


def kernel(X, masks, Wq, Wk, Wv):
    X = np.asarray(X, dtype=np.float32)
    masks = np.asarray(masks)
    in_maps, cfg = make_in_maps(X, masks, np.asarray(Wq, np.float32),
                                np.asarray(Wk, np.float32),
                                np.asarray(Wv, np.float32))
    res = run(in_maps, cfg)
    out = np.empty((B, S, D), np.float32)
    for c in range(N_CORES):
        b, h = c // 2, c % 2
        for k, g in enumerate(OWN_TILES[h]):
            out[b, g * P:(g + 1) * P, :] = res.results[c]["out"][
                k * P:(k + 1) * P, :].astype(np.float32)
    return out


# revision 11
# speedup vs baseline: 1.2436x; 1.2436x over previous
"""Bass/Trainium2 kernel for nn_Attention_75007308857927 (v3).

Reference computation (B=4, S=2048, D=1024):
    Q = X @ Wq.T ; K = X @ Wk.T ; V = X @ Wv.T         (per batch)
    Qn, Kn = row-normalized Q, K
    scores = (Qn @ Kn.T) * m      m[i,j] = 1 if (j > i) or masks[j]==0 else 0
    out = scores @ V

Sharding: 2 cores per batch; each core owns 1024 reordered keys (padded
asc ++ valid asc, host-side) and 1024 queries (OWN_TILES interleave).
KT and V' are pair-AllGathered; scores/out computed per own queries with
anti-causal block skipping (bounds derived from the actual masks,
SPMD-uniform).

v3 changes over v2 (243us -> target ~195us):
  * DMA issue rebalanced around two measured facts: a dma_start costs
    ~0.6-0.8us of sequencer time, and an engine issuing a large DMA
    queue is credit-blocked until most of it drains. The scalar (ACT)
    engine now carries NO early DMAs so the projection squares start on
    time; gpsimd carries xk/wvt/xq + all cc stores + doorbells; sync
    carries wkt/xk-hi/wqt/masks + gather loads.
  * k/q sumsq: squares on ACT, et-chunk accumulation on DVE (sequential
    bf16 adds), so the PE does only 2 ones-matmuls + 8 transposes per
    tensor (was 16 ones-matmuls each, and they stalled the in-order PE
    stream behind a credit-blocked scalar engine).
  * masks shipped as a single fp8 (0/1-exact) tile, one DMA.
  * output stored bf16 (host converts to f32); halves the tail store.

bf16 matmul operands, f32 PSUM accumulation.
"""

import numpy as np
import ml_dtypes

B, S, D = 4, 2048, 1024
HALF = S // 2          # queries / keys per core
N_CORES = 8
P = 128
DC = D // P            # 8 contraction chunks over d
ET = D // P            # 8 e-tiles
JT = S // P            # 16 global key tiles
JTH = HALF // P        # 8 own key tiles

BF16 = ml_dtypes.bfloat16
FP8 = ml_dtypes.float8_e4m3

# query-tile assignment: balanced anti-causal load, sorted ascending
OWN_TILES = [
    [0, 2, 4, 6, 9, 11, 13, 15],   # h = 0
    [1, 3, 5, 7, 8, 10, 12, 14],   # h = 1
]

_CACHE = {}


def _emit(ctx, tc, cfg, xq, xk, wkt, wvt, wqt, maskt, out, cc):
    from concourse import mybir

    JA, TCOPY, GSTART = cfg
    FSTART = (min(GSTART[0:4]), min(GSTART[4:8]))
    f_tiles = [list(range(JA)) + list(range(FSTART[s], JT)) for s in range(2)]
    st_pos = [{t: i for i, t in enumerate(f_tiles[s])} for s in range(2)]
    NT = [len(f_tiles[s]) for s in range(2)]
    mrow = {}
    r = 0
    for s in range(2):
        for t in f_tiles[s]:
            if t >= TCOPY:
                mrow[(s, t)] = r
                r += 1
    NM = r

    nc = tc.nc
    dtb = mybir.dt.bfloat16
    dtf = mybir.dt.float32
    dt8 = mybir.dt.float8e4

    # ---- SBUF pools -------------------------------------------------------
    big_p = ctx.enter_context(tc.tile_pool(name="big", bufs=4))
    xq_p = ctx.enter_context(tc.tile_pool(name="xq", bufs=1))
    qt_p = ctx.enter_context(tc.tile_pool(name="qt", bufs=1))
    kt_p = ctx.enter_context(tc.tile_pool(name="kt", bufs=1))
    vp_p = ctx.enter_context(tc.tile_pool(name="vp", bufs=1))
    row_p = ctx.enter_context(tc.tile_pool(name="rows", bufs=1))
    sq_p = ctx.enter_context(tc.tile_pool(name="sq", bufs=9))
    red_p = ctx.enter_context(tc.tile_pool(name="red", bufs=2))
    stg_p = ctx.enter_context(tc.tile_pool(name="stg", bufs=6))
    mk_p = ctx.enter_context(tc.tile_pool(name="mk", bufs=1))
    ev_p = ctx.enter_context(tc.tile_pool(name="ev", bufs=4))
    # 7 main matmul banks + 1 bank for the row/col reductions
    ps_p = ctx.enter_context(tc.tile_pool(name="psmm", bufs=7, space="PSUM"))
    psr_p = ctx.enter_context(tc.tile_pool(name="psrow", bufs=1, space="PSUM"))

    wkt_s = big_p.tile([P, DC * D], dtb, tag="big")    # [d%128, dc*1024+e]
    wvt_s = big_p.tile([P, DC * D], dtb, tag="big")
    wqt_s = big_p.tile([P, DC * D], dtb, tag="big")
    xk_s = big_p.tile([P, DC * HALF], dtb, tag="big")  # [d%128, dc*1024+p']
    xq_s = xq_p.tile([P, DC * HALF], dtb, tag="xq")    # [d%128, dc*1024+i]
    qt_s = qt_p.tile([P, ET * HALF], dtb, tag="qt")    # [e%128, et*1024+i]
    kt_s = kt_p.tile([P, ET * S], dtb, tag="kt")       # [e%128, et*2048+p]
    vp_s = vp_p.tile([P, JT * D], dtb, tag="vp")       # [p%128, pt*1024+d]
    mk_s = mk_p.tile([P, max(NM, 1) * 512], dt8, tag="mk")

    ones_b = row_p.tile([P, 1], dtb, tag="ones_b")
    ones_f = row_p.tile([1, 1], dtb, tag="ones_f")
    ksq_row = row_p.tile([1, HALF], dtb, tag="sqrow")
    qsq_row = row_p.tile([1, HALF], dtb, tag="sqrow2")
    ksq_col = row_p.tile([P, JTH], dtf, tag="ksqc")
    krec_col = row_p.tile([P, JTH], dtf, tag="krecc")
    kinv_col = row_p.tile([P, JTH], dtf, tag="kinvc")
    qsq_col = row_p.tile([P, ET], dtf, tag="qsqc")
    qrec_col = row_p.tile([P, ET], dtf, tag="qrecc")
    qinv_col = row_p.tile([P, ET], dtf, tag="qinvc")

    warm = row_p.tile([P, 512], dtb, tag="warm")
    nc.vector.memset(ones_b[:], 1.0)
    nc.vector.memset(ones_f[:], 1.0)
    nc.vector.memset(warm[:], 0.5)

    groups = [[0, 1], [2, 3], [4, 5], [6, 7]]

    # PE warm-up: ramp out of the low p-state while input DMAs land
    ps_warm = psr_p.tile([1, 512], mybir.dt.float32, tag="psrow", name="warm")
    for _ in range(10):
        nc.tensor.matmul(ps_warm[:], lhsT=ones_b[:], rhs=warm[:],
                         start=True, stop=True)

    # ---- input DMAs -------------------------------------------------------
    # sync: wkt and xk j5=0 interleaved in wave-1 consumption order (ring
    #       FIFO = issue order), then xk j5=1; later kt-gather loads + even
    #       out stores. gpsimd/scalar issue nothing before ~25us so these
    #       descriptors are never queued behind lower-priority bytes.
    wkt_s3 = wkt_s[:].rearrange("p (dc e) -> p dc e", dc=DC)
    wkt3d = wkt.rearrange("(dc p) e -> p dc e", p=P)
    xk_s4 = xk_s[:].rearrange("p (dc h j) -> p dc h j", dc=DC, h=2)
    xk4d = xk.rearrange("(dc p) (h j) -> p dc h j", p=P, h=2)
    nc.sync.dma_start(wkt_s3[:, 0, 0:256], wkt3d[:, 0, 0:256])
    nc.sync.dma_start(xk_s4[:, 0:2, 0, :], xk4d[:, 0:2, 0, :])
    nc.sync.dma_start(wkt_s3[:, 0, 256:1024], wkt3d[:, 0, 256:1024])
    nc.sync.dma_start(wkt_s3[:, 1, :], wkt3d[:, 1, :])
    nc.sync.dma_start(xk_s4[:, 2:4, 0, :], xk4d[:, 2:4, 0, :])
    nc.sync.dma_start(wkt_s3[:, 2, :], wkt3d[:, 2, :])
    nc.sync.dma_start(wkt_s3[:, 3, :], wkt3d[:, 3, :])
    nc.sync.dma_start(xk_s4[:, 4:6, 0, :], xk4d[:, 4:6, 0, :])
    nc.sync.dma_start(wkt_s3[:, 4, :], wkt3d[:, 4, :])
    nc.sync.dma_start(wkt_s3[:, 5, :], wkt3d[:, 5, :])
    nc.sync.dma_start(xk_s4[:, 6:8, 0, :], xk4d[:, 6:8, 0, :])
    nc.sync.dma_start(wkt_s3[:, 6, :], wkt3d[:, 6, :])
    nc.sync.dma_start(wkt_s3[:, 7, :], wkt3d[:, 7, :])
    for dc in range(0, DC, 2):
        nc.sync.dma_start(xk_s4[:, dc:dc + 2, 1, :], xk4d[:, dc:dc + 2, 1, :])

    wvt3d = wvt.rearrange("(dc p) e -> p dc e", p=P)
    wvt_s3 = wvt_s[:].rearrange("p (dc e) -> p dc e", dc=DC)
    wqt3d = wqt.rearrange("(dc p) e -> p dc e", p=P)
    wqt_s3 = wqt_s[:].rearrange("p (dc e) -> p dc e", dc=DC)
    mk3 = mk_s[:].rearrange("p (r i) -> p r i", r=max(NM, 1))
    m3 = maskt.rearrange("(r p) i -> p r i", p=P)

    # ---- phase B: KT = Wk^T X for own keys + k sumsq ----------------------
    sq_k = []

    def b_evict(j5, et, ps):
        stg = stg_p.tile([P, 512], dtb, tag="stg", name=f"stgb{j5}_{et}")
        nc.vector.tensor_copy(stg[:], ps[:])
        nc.gpsimd.dma_start(cc[f"kt_own{j5}"][et * P:(et + 1) * P, :], stg[:])
        sq = sq_p.tile([P, 512], dtb, tag="sq", name=f"sqb{j5}_{et}")
        nc.scalar.square(sq[:], stg[:])
        sq_k.append(sq)

    acc_k = [red_p.tile([P, 512], dtb, tag="red", name=f"acck{j5}")
             for j5 in range(2)]

    def b_wave(j5):
        # et0-6 dc-outer across 7 banks, then et7 dc-inner
        ps_wave = [ps_p.tile([P, 512], dtf, tag="psmm", name=f"bw{j5}_{et}")
                   for et in range(7)]
        for dc in range(DC):
            for et in range(7):
                nc.tensor.matmul(
                    ps_wave[et][:],
                    lhsT=wkt_s[:, dc * D + et * P: dc * D + (et + 1) * P],
                    rhs=xk_s[:, dc * HALF + j5 * 512: dc * HALF + j5 * 512 + 512],
                    start=(dc == 0), stop=(dc == DC - 1),
                )
        for et in range(7):
            b_evict(j5, et, ps_wave[et])
        ps7 = ps_p.tile([P, 512], dtf, tag="psmm", name=f"bx{j5}")
        for dc in range(DC):
            nc.tensor.matmul(
                ps7[:],
                lhsT=wkt_s[:, dc * D + 7 * P: dc * D + 8 * P],
                rhs=xk_s[:, dc * HALF + j5 * 512: dc * HALF + j5 * 512 + 512],
                start=(dc == 0), stop=(dc == DC - 1),
            )
        b_evict(j5, 7, ps7)
        # DVE sumsq accumulation over the 8 et chunks
        base = 8 * j5
        nc.vector.tensor_add(acc_k[j5][:], sq_k[base][:], sq_k[base + 1][:])
        for et in range(2, 8):
            nc.vector.tensor_add(acc_k[j5][:], acc_k[j5][:], sq_k[base + et][:])

    b_wave(0)
    nc.gpsimd.collective_compute(
        "AllGather", mybir.AluOpType.bypass, replica_groups=groups,
        ins=[cc["kt_own0"][:]], outs=[cc["kt_gath0"][:]])
    # wvt now: gpsimd's stream reaches this ~25us in (after the j5=0 kt
    # stores), so wvt's 2MB never contends with the wave-1/2 inputs
    nc.gpsimd.dma_start(wvt_s3[:, 0:4, :], wvt3d[:, 0:4, :])
    nc.gpsimd.dma_start(wvt_s3[:, 4:8, :], wvt3d[:, 4:8, :])
    b_wave(1)
    nc.gpsimd.collective_compute(
        "AllGather", mybir.AluOpType.bypass, replica_groups=groups,
        ins=[cc["kt_own1"][:]], outs=[cc["kt_gath1"][:]])

    # xq on gpsimd (needed only by phase E); wqt + masks on scalar, whose
    # stream reaches this point only after the wave-1 squares (~35us)
    xq3d = xq.rearrange("(dc p) e -> p dc e", p=P)
    xq_s3 = xq_s[:].rearrange("p (dc e) -> p dc e", dc=DC)
    nc.gpsimd.dma_start(xq_s3[:, 0:4, :], xq3d[:, 0:4, :])
    nc.gpsimd.dma_start(xq_s3[:, 4:8, :], xq3d[:, 4:8, :])
    nc.scalar.dma_start(wqt_s3[:, 0:4, :], wqt3d[:, 0:4, :])
    nc.scalar.dma_start(wqt_s3[:, 4:8, :], wqt3d[:, 4:8, :])
    nc.scalar.dma_start(mk3[:, :, :], m3[:, :, :])

    # k sumsq -> kinv: 2 ones-matmuls + 8 transposes on PE, recip on DVE,
    # sqrt on ACT
    for j5 in range(2):
        pr = psr_p.tile([1, 512], dtf, tag="psrow", name=f"ksqps{j5}")
        nc.tensor.matmul(pr[:], lhsT=ones_b[:], rhs=acc_k[j5][:],
                         start=True, stop=True)
        nc.vector.tensor_copy(ksq_row[0:1, j5 * 512:(j5 + 1) * 512], pr[:])
    for c in range(JTH):
        pc = psr_p.tile([P, 1], dtf, tag="psrow", name=f"kpc{c}")
        nc.tensor.matmul(pc[:], lhsT=ksq_row[0:1, c * P:(c + 1) * P],
                         rhs=ones_f[:], start=True, stop=True)
        nc.vector.tensor_copy(ksq_col[:, c:c + 1], pc[:])
    nc.vector.reciprocal(krec_col[:], ksq_col[:])
    nc.scalar.sqrt(kinv_col[:], krec_col[:])

    # ---- phase D: V' = V * kinv for own keys ------------------------------
    for jt in range(JTH):
        ps_a = ps_p.tile([P, 512], dtf, tag="psmm")
        ps_b = ps_p.tile([P, 512], dtf, tag="psmm")
        pspair = [ps_a, ps_b]
        for dc in range(DC):
            for e5 in range(2):
                nc.tensor.matmul(
                    pspair[e5][:],
                    lhsT=xk_s[:, dc * HALF + jt * P: dc * HALF + (jt + 1) * P],
                    rhs=wvt_s[:, dc * D + e5 * 512: dc * D + e5 * 512 + 512],
                    start=(dc == 0), stop=(dc == DC - 1),
                )
        half = jt // 4
        for e5 in range(2):
            stg = stg_p.tile([P, 512], dtb, tag="stg", name=f"stgv{jt}_{e5}")
            nc.vector.tensor_scalar_mul(stg[:], pspair[e5][:],
                                        kinv_col[:, jt:jt + 1])
            nc.gpsimd.dma_start(
                cc[f"v_own{half}"][(jt % 4) * P:(jt % 4 + 1) * P,
                                   e5 * 512: e5 * 512 + 512], stg[:])
        if jt == 3:
            nc.gpsimd.collective_compute(
                "AllGather", mybir.AluOpType.bypass, replica_groups=groups,
                ins=[cc["v_own0"][:]], outs=[cc["v_gath0"][:]])
    nc.gpsimd.collective_compute(
        "AllGather", mybir.AluOpType.bypass, replica_groups=groups,
        ins=[cc["v_own1"][:]], outs=[cc["v_gath1"][:]])

    # kt SBUF loads (1MB per (half, rank)) on sync
    kt3 = kt_s[:].rearrange("p (et j) -> p et j", et=ET, j=S)
    for h5 in range(2):
        gath = cc[f"kt_gath{h5}"]
        for r in range(2):
            src3 = gath[r].rearrange("(et p) j -> p et j", p=P)
            dst = kt3[:, :, r * HALF + h5 * 512: r * HALF + h5 * 512 + 512]
            nc.sync.dma_start(dst, src3)

    # vp SBUF loads on gpsimd
    vp3 = vp_s[:].rearrange("p (jtl e) -> p jtl e", jtl=JT, e=D)
    for h5 in range(2):
        gath = cc[f"v_gath{h5}"]
        for r in range(2):
            src_ap = gath[r].rearrange("(jtl p) e -> p jtl e", p=P)
            dst = vp3[:, r * JTH + h5 * 4: r * JTH + h5 * 4 + 4, :]
            nc.gpsimd.dma_start(dst, src_ap)

    # ---- phase E: QT = Wq^T X for own queries + q sumsq -------------------
    sq_q = []

    def e_evict(i5, et, ps):
        qtsl = qt_s[:, et * HALF + i5 * 512: et * HALF + i5 * 512 + 512]
        nc.vector.tensor_copy(qtsl, ps[:])
        sq = sq_p.tile([P, 512], dtb, tag="sq", name=f"sqe{i5}_{et}")
        nc.scalar.square(sq[:], qtsl)
        sq_q.append(sq)

    acc_q = [red_p.tile([P, 512], dtb, tag="red", name=f"accq{i5}")
             for i5 in range(2)]

    def e_wave(i5):
        ps_wave = [ps_p.tile([P, 512], dtf, tag="psmm", name=f"ew{i5}_{et}")
                   for et in range(7)]
        for dc in range(DC):
            for et in range(7):
                nc.tensor.matmul(
                    ps_wave[et][:],
                    lhsT=wqt_s[:, dc * D + et * P: dc * D + (et + 1) * P],
                    rhs=xq_s[:, dc * HALF + i5 * 512: dc * HALF + i5 * 512 + 512],
                    start=(dc == 0), stop=(dc == DC - 1),
                )
        for et in range(7):
            e_evict(i5, et, ps_wave[et])
        ps7 = ps_p.tile([P, 512], dtf, tag="psmm", name=f"ex{i5}")
        for dc in range(DC):
            nc.tensor.matmul(
                ps7[:],
                lhsT=wqt_s[:, dc * D + 7 * P: dc * D + 8 * P],
                rhs=xq_s[:, dc * HALF + i5 * 512: dc * HALF + i5 * 512 + 512],
                start=(dc == 0), stop=(dc == DC - 1),
            )
        e_evict(i5, 7, ps7)
        base = 8 * i5
        nc.vector.tensor_add(acc_q[i5][:], sq_q[base][:], sq_q[base + 1][:])
        for et in range(2, 8):
            nc.vector.tensor_add(acc_q[i5][:], acc_q[i5][:], sq_q[base + et][:])

    e_wave(0)
    e_wave(1)

    # q sumsq -> qinv (qinv only needed by phase G evictions)
    for i5 in range(2):
        pr = psr_p.tile([1, 512], dtf, tag="psrow", name=f"qsqps{i5}")
        nc.tensor.matmul(pr[:], lhsT=ones_b[:], rhs=acc_q[i5][:],
                         start=True, stop=True)
        nc.vector.tensor_copy(qsq_row[0:1, i5 * 512:(i5 + 1) * 512], pr[:])
    for c in range(ET):
        pc = psr_p.tile([P, 1], dtf, tag="psrow", name=f"qpc{c}")
        nc.tensor.matmul(pc[:], lhsT=qsq_row[0:1, c * P:(c + 1) * P],
                         rhs=ones_f[:], start=True, stop=True)
        nc.vector.tensor_copy(qsq_col[:, c:c + 1], pc[:])
    nc.vector.reciprocal(qrec_col[:], qsq_col[:])
    nc.scalar.sqrt(qinv_col[:], qrec_col[:])

    # ---- phase F: score blocks (dense padded part + anti-causal part) -----
    st_blks = [big_p.tile([P, NT[s] * 512], dtb, tag="big", name=f"st_blk{s}")
               for s in range(2)]
    # consume the gather-0 half (local tiles 0-3, 8-11) first: its SBUF
    # loads land ~20us before gather-1's
    f_order = [sorted(f_tiles[s], key=lambda t: ((t // 4) % 2, t))
               for s in range(2)]
    for s in range(2):
        st_blk = st_blks[s]
        for t in f_order[s]:
            pos = st_pos[s][t]
            ps = ps_p.tile([P, 512], dtf, tag="psmm")
            for et in range(ET):
                nc.tensor.matmul(
                    ps[:],
                    lhsT=kt_s[:, et * S + t * P: et * S + (t + 1) * P],
                    rhs=qt_s[:, et * HALF + s * 512: et * HALF + s * 512 + 512],
                    start=(et == 0), stop=(et == ET - 1),
                )
            dst = st_blk[:, pos * 512:(pos + 1) * 512]
            if t < TCOPY:
                nc.vector.tensor_copy(dst, ps[:])
            else:
                rr = mrow[(s, t)]
                nc.vector.tensor_mul(dst, ps[:], mk3[:, rr, :])

    # ---- phase G: out = (ST^T @ V') * qinv, stored bf16 -------------------
    for s in range(2):
        st_blk = st_blks[s]
        for c in range(4):
            k = 4 * s + c
            g_tiles = list(range(JA)) + list(range(GSTART[k], JT))
            # accumulate v_gath0's tiles first so the chain only stalls on
            # the late v_gath1 loads near its end
            g_tiles.sort(key=lambda t: ((t // 4) % 2, t))
            ev = ev_p.tile([P, D], dtb, tag="ev", name=f"ev{k}")
            for d5 in range(2):
                ps = ps_p.tile([P, 512], dtf, tag="psmm")
                for n, t in enumerate(g_tiles):
                    pos = st_pos[s][t]
                    nc.tensor.matmul(
                        ps[:],
                        lhsT=st_blk[:, pos * 512 + c * P: pos * 512 + (c + 1) * P],
                        rhs=vp_s[:, t * D + d5 * 512: t * D + d5 * 512 + 512],
                        start=(n == 0), stop=(n == len(g_tiles) - 1),
                    )
                nc.vector.tensor_scalar_mul(ev[:, d5 * 512:d5 * 512 + 512],
                                            ps[:], qinv_col[:, k:k + 1])
            eng = nc.sync if k % 2 == 0 else nc.scalar
            eng.dma_start(out[k * P:(k + 1) * P, :], ev[:])


def _build(cfg):
    if cfg in _CACHE:
        return _CACHE[cfg]
    import concourse.tile as tile
    from concourse import bacc, mybir

    JA, TCOPY, GSTART = cfg
    FSTART = (min(GSTART[0:4]), min(GSTART[4:8]))
    f_tiles = [list(range(JA)) + list(range(FSTART[s], JT)) for s in range(2)]
    NM = sum(sum(1 for t in f_tiles[s] if t >= TCOPY) for s in range(2))

    dtb = mybir.dt.bfloat16
    dt8 = mybir.dt.float8e4
    nc = bacc.Bacc("TRN2", target_bir_lowering=False, debug=False,
                   enable_asserts=True, num_devices=N_CORES)
    xq = nc.dram_tensor("xq", [D, HALF], dtb, kind="ExternalInput").ap()
    xk = nc.dram_tensor("xk", [D, HALF], dtb, kind="ExternalInput").ap()
    wkt = nc.dram_tensor("wkt", [D, D], dtb, kind="ExternalInput").ap()
    wvt = nc.dram_tensor("wvt", [D, D], dtb, kind="ExternalInput").ap()
    wqt = nc.dram_tensor("wqt", [D, D], dtb, kind="ExternalInput").ap()
    maskt = nc.dram_tensor("maskt", [max(NM, 1) * P, 512], dt8,
                           kind="ExternalInput").ap()
    out = nc.dram_tensor("out", [HALF, D], dtb, kind="ExternalOutput").ap()
    cc = {}
    for h5 in range(2):
        cc[f"kt_own{h5}"] = nc.dram_tensor(f"kt_own{h5}", [D, 512], dtb).ap()
        cc[f"kt_gath{h5}"] = nc.dram_tensor(f"kt_gath{h5}", [2, D, 512], dtb).ap()
        cc[f"v_own{h5}"] = nc.dram_tensor(f"v_own{h5}", [512, D], dtb).ap()
        cc[f"v_gath{h5}"] = nc.dram_tensor(f"v_gath{h5}", [2, 512, D], dtb).ap()

    from contextlib import ExitStack
    with tile.TileContext(nc) as tc:
        with ExitStack() as ctx:
            _emit(ctx, tc, cfg, xq, xk, wkt, wvt, wqt, maskt, out, cc)
    nc.compile()
    _CACHE[cfg] = nc
    return nc


def plan(masks):
    """Derive key reorder + skip bounds from the masks (SPMD-uniform)."""
    masks = np.asarray(masks)
    packed, nAs = [], []
    for b in range(B):
        iA = np.flatnonzero(masks[b] == 0)   # padded: always visible
        iV = np.flatnonzero(masks[b] != 0)   # valid: visible iff j > i
        packed.append(np.concatenate([iA, iV]).astype(np.int64))
        nAs.append(len(iA))
    JA = max(max(-(-n // P) for n in nAs), 1)
    TCOPY = min(n // P for n in nAs)
    GSTART = []
    for k in range(8):
        st = JT
        for b in range(B):
            pk, nA = packed[b], nAs[b]
            for h in range(2):
                g = OWN_TILES[h][k]
                t = JA
                while t < JT:
                    p = np.arange(t * P, (t + 1) * P)
                    mv = np.where(p >= nA, pk[p], -1).max()
                    if mv > g * P:
                        break
                    t += 1
                st = min(st, t)
        GSTART.append(st)
    # slots ascend in tile index -> bounds must be non-increasing suffixes
    for k in range(6, -1, -1):
        GSTART[k] = min(GSTART[k], GSTART[k + 1])
    return (JA, TCOPY, tuple(GSTART)), packed, nAs


def make_in_maps(X, masks, Wq, Wk, Wv):
    """Host-side key reorder + layout: one input map per core."""
    cfg, packed, nAs = plan(masks)
    JA, TCOPY, GSTART = cfg
    FSTART = (min(GSTART[0:4]), min(GSTART[4:8]))
    f_tiles = [list(range(JA)) + list(range(FSTART[s], JT)) for s in range(2)]
    NM = sum(sum(1 for t in f_tiles[s] if t >= TCOPY) for s in range(2))
    in_maps = []
    wkt_h = np.ascontiguousarray(Wk.T).astype(BF16)
    wvt_h = np.ascontiguousarray(Wv.T).astype(BF16)
    wqt_h = np.ascontiguousarray(Wq.T).astype(BF16)
    for c in range(N_CORES):
        b, h = c // 2, c % 2
        XT = X[b].T.astype(BF16)                                # [D, S]
        pk, nA = packed[b], nAs[b]
        own_keys = pk[h * HALF:(h + 1) * HALF]
        tiles = OWN_TILES[h]
        qrows = np.concatenate([np.arange(g * P, (g + 1) * P) for g in tiles])
        # mask blocks, [NM*128, 512]: rows = global reordered key pos,
        # cols = own queries in slot order
        mt = np.zeros((max(NM, 1) * P, 4 * P), FP8)
        cols = [np.concatenate([np.arange(tiles[4 * s + cc] * P,
                                          (tiles[4 * s + cc] + 1) * P)
                                for cc in range(4)]) for s in range(2)]
        r = 0
        for s in range(2):
            for t in f_tiles[s]:
                if t < TCOPY:
                    continue
                p = t * P + np.arange(P)
                is_pad = p < nA
                vis = is_pad[:, None] | (pk[p][:, None] > cols[s][None, :])
                mt[r * P:(r + 1) * P, :] = vis
                r += 1
        in_maps.append({
            "xq": np.ascontiguousarray(XT[:, qrows]),
            "xk": np.ascontiguousarray(XT[:, own_keys]),
            "wkt": wkt_h,
            "wvt": wvt_h,
            "wqt": wqt_h,
            "maskt": mt,
        })
    return in_maps, cfg


def run(in_maps, cfg, **kw):
    from concourse.bass_utils import run_bass_kernel_spmd
    nc = _build(cfg)
    return run_bass_kernel_spmd(nc, in_maps, list(range(N_CORES)), **kw)


def kernel(X, masks, Wq, Wk, Wv):
    X = np.asarray(X, dtype=np.float32)
    masks = np.asarray(masks)
    in_maps, cfg = make_in_maps(X, masks, np.asarray(Wq, np.float32),
                                np.asarray(Wk, np.float32),
                                np.asarray(Wv, np.float32))
    res = run(in_maps, cfg)
    out = np.empty((B, S, D), np.float32)
    for c in range(N_CORES):
        b, h = c // 2, c % 2
        for k, g in enumerate(OWN_TILES[h]):
            out[b, g * P:(g + 1) * P, :] = res.results[c]["out"][
                k * P:(k + 1) * P, :].astype(np.float32)
    return out
